# revision 12
# baseline (speedup 1.0000x reference)
"""Cross-attention Trainium2 kernel (nn_CrossAttention, B=2, L=2048, D=1024,
Dctx=768, 16 heads x 64).

Sharding: 8 cores = 2 (batch) x 4 (head-groups of 4 heads). Each core computes
its batch's Q/K/V projections for its 4 heads, flash-style attention in the
transposed (S^T) domain, and a partial output projection; the host sums the
head-group partials and adds b_o.

All activations live transposed on-chip (xT, ctxT, qT, kT, attnT) so every
matmul contracts over the partition dim with no on-chip transposes; operands
are fp16 (full PE streaming rate) with fp32 PSUM accumulation. Heads are
processed in pairs: the pair's scores matmuls contract K=64 on PE row-groups
(0,0) and (64,0) and stream CONCURRENTLY into the two banks of one [128,1024]
PSUM tile, so a head-pair's scores cost one stream instead of two. One
1024-wide exp covers both heads. V tiles are padded to 128 columns (64 v + 32
ones for the softmax denominator + 32 zeros) so every stationary load takes
the fast-weight-load path. Output partials are fp16, summed on the host.
"""
import numpy as np

import concourse.bass as bass
import concourse.tile as tile
from concourse import bacc, mybir, bass_utils

F16 = mybir.dt.float16
F32 = mybir.dt.float32
EXP = mybir.ActivationFunctionType.Exp
IDENT = mybir.ActivationFunctionType.Identity

# Problem shape (hardcoded per harness contract)
B, LQ, D = 2, 2048, 1024
DCTX = 768
NH, HD = 16, 64
SCALE = 1.0 / 8.0  # 1/sqrt(64)

# Per-core shard: 4 heads (one group), one batch
GH = 4                # heads per core
ONES = 32             # d-replication rows per head
VW = 128              # per-head v_t width: 64 v + 32 ones + 32 zero pad (FWL)
VAW = GH * VW         # 512
GD = GH * HD          # 256: real v columns per chunk
KT_Q = D // 128       # 8
KT_C = DCTX // 128    # 6
NLK = LQ // 128       # 16 key tiles
NS = LQ // 512        # 4 query 512-slices
HALF = 1024


def _build():
    nc = bacc.Bacc("TRN2", target_bir_lowering=False, debug=False,
                   enable_asserts=False, num_devices=8)

    xT_d = nc.dram_tensor("xT", (D, LQ), F16, kind="ExternalInput").ap()
    cT_d = nc.dram_tensor("ctxT", (DCTX, LQ), F16, kind="ExternalInput").ap()
    wq_d = nc.dram_tensor("wq", (D, 256), F16, kind="ExternalInput").ap()
    wk_d = nc.dram_tensor("wk", (DCTX, 256), F16, kind="ExternalInput").ap()
    wv_d = nc.dram_tensor("wv", (DCTX, GD), F16, kind="ExternalInput").ap()
    wo_d = nc.dram_tensor("wo", (256, D), F16, kind="ExternalInput").ap()
    bq_d = nc.dram_tensor("bq", (128, 2), F32, kind="ExternalInput").ap()
    bk_d = nc.dram_tensor("bk", (128, 2), F32, kind="ExternalInput").ap()
    bvb_d = nc.dram_tensor("bvb", (128, GD), F32, kind="ExternalInput").ap()
    out_d = nc.dram_tensor("outT", (D, LQ), F16, kind="ExternalOutput").ap()

    with tile.TileContext(nc) as tc:
        with tc.tile_pool(name="w", bufs=1) as wp, \
             tc.tile_pool(name="xt", bufs=10) as xtp, \
             tc.tile_pool(name="ct", bufs=24) as ctp, \
             tc.tile_pool(name="act", bufs=1) as actp, \
             tc.tile_pool(name="expp", bufs=3) as expp, \
             tc.tile_pool(name="scrp", bufs=3) as scrp, \
             tc.tile_pool(name="rdp", bufs=3) as rdp, \
             tc.tile_pool(name="outp", bufs=3) as outp, \
             tc.tile_pool(name="ps_s", bufs=2, space="PSUM") as ps_s, \
             tc.tile_pool(name="ps_w", bufs=4, space="PSUM") as ps_w:

            # ---- weight/bias tiles (DMAs issued interleaved below) ----
            wq_t = wp.tile([128, KT_Q * 256], F16, tag="wq")
            wk_t = wp.tile([128, KT_C * 256], F16, tag="wk")
            wv_t = wp.tile([128, KT_C * GD], F16, tag="wv")
            wo_t = wp.tile([128, 2 * D], F16, tag="wo")
            bq_t = wp.tile([128, 2], F32, tag="bq")
            bk_t = wp.tile([128, 2], F32, tag="bk")
            bvb_t = wp.tile([128, GD], F32, tag="bvb")

            # K proj needs these first
            nc.sync.dma_start(wk_t[:].rearrange("p (kt m) -> p kt m", m=256),
                              wk_d.rearrange("(kt p) m -> p kt m", p=128))
            nc.sync.dma_start(bk_t[:], bk_d[:])

            # ---- persistent activation tiles ----
            qT = [actp.tile([128, LQ], F16, tag=f"qT{p}", name=f"qT{p}")
                  for p in range(2)]
            kT = [actp.tile([128, LQ], F16, tag=f"kT{p}", name=f"kT{p}")
                  for p in range(2)]
            v_t = actp.tile([128, NLK * VAW], F16, tag="v")
            aT = [actp.tile([128, LQ], F16, tag=f"aT{p}", name=f"aT{p}")
                  for p in range(2)]

            # constant ones (softmax denominator) / zero-pad rows of v_t
            v4 = v_t[:].rearrange("p (j w) -> p j w", w=VAW)
            for h in range(GH):
                nc.vector.memset(v4[:, :, VW * h + HD:VW * h + HD + ONES], 1.0)
                nc.vector.memset(v4[:, :, VW * h + HD + ONES:VW * (h + 1)], 0.0)

            # ---- K projection + interleaved weight DMAs, per 512-slice ----
            ct_tiles = {}
            for s in range(NS):
                for kt in range(KT_C):
                    t = ctp.tile([128, 512], F16, tag="ct")
                    nc.sync.dma_start(
                        t[:], cT_d[128 * kt:128 * (kt + 1), 512 * s:512 * (s + 1)])
                    ct_tiles[(kt, s)] = t
                for p in range(2):
                    ps = ps_w.tile([128, 512], F32, tag="mm")
                    for kt in range(KT_C):
                        nc.tensor.matmul(
                            ps[:], wk_t[:, 256 * kt + 128 * p:256 * kt + 128 * (p + 1)],
                            ct_tiles[(kt, s)][:],
                            start=(kt == 0), stop=(kt == KT_C - 1))
                    nc.scalar.activation(
                        kT[p][:, 512 * s:512 * (s + 1)], ps[:], IDENT,
                        bias=bk_t[:, p:p + 1])
                # stagger the remaining weight loads behind the ct slices
                if s == 0:
                    nc.sync.dma_start(
                        wv_t[:].rearrange("p (kt m) -> p kt m", m=GD),
                        wv_d.rearrange("(kt p) m -> p kt m", p=128))
                    nc.sync.dma_start(bvb_t[:], bvb_d[:])
                elif s == 1:
                    nc.sync.dma_start(
                        wq_t[:].rearrange("p (kt m) -> p kt m", m=256),
                        wq_d.rearrange("(kt p) m -> p kt m", p=128))
                    nc.sync.dma_start(bq_t[:], bq_d[:])
                elif s == 2:
                    nc.sync.dma_start(
                        wo_t[:].rearrange("p (p2 m) -> p p2 m", m=1024),
                        wo_d.rearrange("(p2 p) m -> p p2 m", p=128))

            # ---- V projection (ctx resident; x DMA still in flight) ----
            for j in range(NLK):
                ps = ps_w.tile([128, 512], F32, tag="mm")
                s, jj = j // 4, j % 4
                for kt in range(KT_C):
                    nc.tensor.matmul(
                        ps[:, 0:GD],
                        ct_tiles[(kt, s)][:, 128 * jj:128 * (jj + 1)],
                        wv_t[:, GD * kt:GD * (kt + 1)],
                        start=(kt == 0), stop=(kt == KT_C - 1))
                for h in range(GH):
                    nc.vector.tensor_add(
                        v_t[:, VAW * j + VW * h:VAW * j + VW * h + HD],
                        ps[:, HD * h:HD * (h + 1)],
                        bvb_t[:, HD * h:HD * (h + 1)])

            # ---- Q projection (both pairs), per 512-slice ----
            for s in range(NS):
                xt_tiles = []
                for kt in range(KT_Q):
                    t = xtp.tile([128, 512], F16, tag="xt")
                    nc.sync.dma_start(
                        t[:], xT_d[128 * kt:128 * (kt + 1), 512 * s:512 * (s + 1)])
                    xt_tiles.append(t)
                for p in range(2):
                    ps = ps_w.tile([128, 512], F32, tag="mm")
                    for kt in range(KT_Q):
                        nc.tensor.matmul(
                            ps[:], wq_t[:, 256 * kt + 128 * p:256 * kt + 128 * (p + 1)],
                            xt_tiles[kt][:],
                            start=(kt == 0), stop=(kt == KT_Q - 1))
                    nc.scalar.activation(
                        qT[p][:, 512 * s:512 * (s + 1)], ps[:], IDENT,
                        bias=bq_t[:, p:p + 1])

            def out_proj(s):
                for mo in range(D // 128):
                    ps = ps_w.tile([128, 512], F32, tag="mm")
                    for p in range(2):
                        nc.tensor.matmul(
                            ps[:], wo_t[:, D * p + 128 * mo:D * p + 128 * (mo + 1)],
                            aT[p][:, 512 * s:512 * (s + 1)],
                            start=(p == 0), stop=(p == 1))
                    ot = outp.tile([128, 512], F16, tag="out")
                    nc.vector.tensor_copy(ot[:], ps[:])
                    nc.sync.dma_start(
                        out_d[128 * mo:128 * (mo + 1), 512 * s:512 * (s + 1)], ot[:])

            # ---- attention, one head-pair at a time ----
            for half in range(2):
                for p in range(2):
                    hA, hB = 2 * p, 2 * p + 1
                    pa = {(hh, n): ps_w.tile([128, 512], F32, tag="mm",
                                             name=f"pa{half}_{p}_{hh}_{n}")
                          for hh in (0, 1) for n in range(2)}
                    for j in range(NLK):
                        ks = slice(128 * j, 128 * (j + 1))
                        for n in range(2):
                            cols = slice(HALF * half + 512 * n,
                                         HALF * half + 512 * (n + 1))
                            st = ps_s.tile([128, HALF], F32, tag="s")
                            # concurrent PE row-group pair: head A rows 0:64,
                            # head B rows 64:128, disjoint PSUM banks
                            nc.tensor.matmul(
                                st[:, 0:512], kT[p][0:64, ks], qT[p][0:64, cols],
                                start=True, stop=True)
                            nc.tensor.matmul(
                                st[:, 512:1024], kT[p][64:128, ks],
                                qT[p][64:128, cols], start=True, stop=True)
                            ex = expp.tile([128, HALF], F16, tag="expS")
                            nc.scalar.activation(ex[:], st[:], EXP, scale=SCALE)
                            nc.tensor.matmul(
                                pa[(0, n)][:],
                                v_t[:, VAW * j + VW * hA:VAW * j + VW * (hA + 1)],
                                ex[:, 0:512],
                                start=(j == 0), stop=(j == NLK - 1))
                            nc.tensor.matmul(
                                pa[(1, n)][:],
                                v_t[:, VAW * j + VW * hB:VAW * j + VW * (hB + 1)],
                                ex[:, 512:1024],
                                start=(j == 0), stop=(j == NLK - 1))
                    # normalize: attnT = attnU * (1/d); d-block replicated to
                    # 64 partitions via PSUM->SBUF shifts (SBUF->SBUF illegal)
                    for hh in (0, 1):
                        r0 = 64 * hh
                        for n in range(2):
                            pan = pa[(hh, n)]
                            dsb = rdp.tile([64, 512], F32, tag="dsb")
                            nc.vector.tensor_copy(dsb[0:32, :], pan[64:96, :])
                            nc.vector.tensor_copy(dsb[32:64, :], pan[64:96, :])
                            scr = scrp.tile([64, 512], F32, tag="scr")
                            nc.vector.tensor_copy(scr[:], pan[0:64, :])
                            rd = rdp.tile([64, 512], F32, tag="rd")
                            rds = rdp.tile([64, 512], F32, tag="rds")
                            nc.vector.reciprocal_approx_accurate(
                                rd[:], dsb[:], rds[:])
                            cols = slice(HALF * half + 512 * n,
                                         HALF * half + 512 * (n + 1))
                            nc.vector.tensor_mul(
                                aT[p][r0:r0 + 64, cols], scr[:], rd[:])
                if half == 0:
                    out_proj(0)
                    out_proj(1)
            out_proj(2)
            out_proj(3)

    nc.compile()
    return nc


_NC_CACHE = []


def _get_nc():
    if not _NC_CACHE:
        _NC_CACHE.append(_build())
    return _NC_CACHE[0]


def kernel_run(inputs, trace=False, **kw):
    """Run on HW; returns (full_output, BassKernelResults)."""
    x = np.asarray(inputs["x"], np.float32)
    context = np.asarray(inputs["context"], np.float32)
    w_q = np.asarray(inputs["w_q"], np.float32)
    b_q = np.asarray(inputs["b_q"], np.float32)
    w_k = np.asarray(inputs["w_k"], np.float32)
    b_k = np.asarray(inputs["b_k"], np.float32)
    w_v = np.asarray(inputs["w_v"], np.float32)
    b_v = np.asarray(inputs["b_v"], np.float32)
    w_o = np.asarray(inputs["w_o"], np.float32)
    b_o = np.asarray(inputs["b_o"], np.float32)

    f16 = np.float16
    xT_h = [np.ascontiguousarray(x[b].T).astype(f16) for b in range(B)]
    cT_h = [np.ascontiguousarray(context[b].T).astype(f16) for b in range(B)]

    maps = []
    for c in range(8):
        b, g = c // 4, c % 4
        hs = slice(256 * g, 256 * (g + 1))
        maps.append({
            "xT": xT_h[b],
            "ctxT": cT_h[b],
            "wq": np.ascontiguousarray(w_q[:, hs]).astype(f16),
            "wk": np.ascontiguousarray(w_k[:, hs]).astype(f16),
            "wv": np.ascontiguousarray(w_v[:, hs]).astype(f16),
            "wo": np.ascontiguousarray(w_o[hs, :]).astype(f16),
            "bq": np.ascontiguousarray(b_q[hs].reshape(2, 128).T),
            "bk": np.ascontiguousarray(b_k[hs].reshape(2, 128).T),
            "bvb": np.ascontiguousarray(
                np.broadcast_to(b_v[None, hs], (128, GD)).astype(np.float32)),
        })

    nc = _get_nc()
    res = bass_utils.run_bass_kernel_spmd(nc, maps, core_ids=list(range(8)),
                                          trace=trace, **kw)
    out = np.empty((B, LQ, D), np.float32)
    for b in range(B):
        acc = res.results[4 * b]["outT"].astype(np.float32)
        for g in range(1, 4):
            acc = acc + res.results[4 * b + g]["outT"].astype(np.float32)
        out[b] = acc.T + b_o[None, :]
    return out, res


def kernel(**inputs) -> np.ndarray:
    out, _ = kernel_run(inputs)
    return out


# revision 13
# speedup vs baseline: 1.0002x; 1.0002x over previous
"""Cross-attention Trainium2 kernel (nn_CrossAttention, B=2, L=2048, D=1024,
Dctx=768, 16 heads x 64).

Sharding: 8 cores = 2 (batch) x 4 (head-groups of 4 heads). Each core computes
its batch's Q/K/V projections for its 4 heads, flash-style attention in the
transposed (S^T) domain, and a partial output projection; the host sums the
head-group partials and adds b_o.

All activations live transposed on-chip (xT, ctxT, qT, kT, attnT) so every
matmul contracts over the partition dim with no on-chip transposes; operands
are fp16 (full PE streaming rate) with fp32 PSUM accumulation. Heads are
processed in pairs: the pair's scores matmuls contract K=64 on PE row-groups
(0,0) and (64,0) and stream CONCURRENTLY into the two banks of one [128,1024]
PSUM tile, so a head-pair's scores cost one stream instead of two. One
1024-wide exp covers both heads. V tiles are padded to 128 columns (64 v + 32
ones for the softmax denominator + 32 zeros) so every stationary load takes
the fast-weight-load path. Output partials are fp16, summed on the host.
"""
import numpy as np

import concourse.bass as bass
import concourse.tile as tile
from concourse import bacc, mybir, bass_utils

F16 = mybir.dt.float16
F32 = mybir.dt.float32
EXP = mybir.ActivationFunctionType.Exp
IDENT = mybir.ActivationFunctionType.Identity

# Problem shape (hardcoded per harness contract)
B, LQ, D = 2, 2048, 1024
DCTX = 768
NH, HD = 16, 64
SCALE = 1.0 / 8.0  # 1/sqrt(64)

# Per-core shard: 4 heads (one group), one batch
GH = 4                # heads per core
ONES = 32             # d-replication rows per head
VW = 128              # per-head v_t width: 64 v + 32 ones + 32 zero pad (FWL)
VAW = GH * VW         # 512
GD = GH * HD          # 256: real v columns per chunk
KT_Q = D // 128       # 8
KT_C = DCTX // 128    # 6
NLK = LQ // 128       # 16 key tiles
NS = LQ // 512        # 4 query 512-slices
HALF = 1024


def _build():
    nc = bacc.Bacc("TRN2", target_bir_lowering=False, debug=False,
                   enable_asserts=False, num_devices=8)

    xT_d = nc.dram_tensor("xT", (D, LQ), F16, kind="ExternalInput").ap()
    cT_d = nc.dram_tensor("ctxT", (DCTX, LQ), F16, kind="ExternalInput").ap()
    wq_d = nc.dram_tensor("wq", (D, 256), F16, kind="ExternalInput").ap()
    wk_d = nc.dram_tensor("wk", (DCTX, 256), F16, kind="ExternalInput").ap()
    wv_d = nc.dram_tensor("wv", (DCTX, GD), F16, kind="ExternalInput").ap()
    wo_d = nc.dram_tensor("wo", (256, D), F16, kind="ExternalInput").ap()
    bq_d = nc.dram_tensor("bq", (128, 2), F32, kind="ExternalInput").ap()
    bk_d = nc.dram_tensor("bk", (128, 2), F32, kind="ExternalInput").ap()
    bvb_d = nc.dram_tensor("bvb", (128, GD), F32, kind="ExternalInput").ap()
    out_d = nc.dram_tensor("outT", (D, LQ), F16, kind="ExternalOutput").ap()

    with tile.TileContext(nc) as tc:
        with tc.tile_pool(name="w", bufs=1) as wp, \
             tc.tile_pool(name="xt", bufs=10) as xtp, \
             tc.tile_pool(name="ct", bufs=24) as ctp, \
             tc.tile_pool(name="act", bufs=1) as actp, \
             tc.tile_pool(name="expp", bufs=3) as expp, \
             tc.tile_pool(name="scrp", bufs=3) as scrp, \
             tc.tile_pool(name="rdp", bufs=3) as rdp, \
             tc.tile_pool(name="outp", bufs=3) as outp, \
             tc.tile_pool(name="ps_s", bufs=2, space="PSUM") as ps_s, \
             tc.tile_pool(name="ps_w", bufs=4, space="PSUM") as ps_w:

            # ---- weight/bias tiles (DMAs issued interleaved below) ----
            wq_t = wp.tile([128, KT_Q * 256], F16, tag="wq")
            wk_t = wp.tile([128, KT_C * 256], F16, tag="wk")
            wv_t = wp.tile([128, KT_C * GD], F16, tag="wv")
            wo_t = wp.tile([128, 2 * D], F16, tag="wo")
            bq_t = wp.tile([128, 2], F32, tag="bq")
            bk_t = wp.tile([128, 2], F32, tag="bk")
            bvb_t = wp.tile([128, GD], F32, tag="bvb")

            # K proj needs these first
            nc.sync.dma_start(wk_t[:].rearrange("p (kt m) -> p kt m", m=256),
                              wk_d.rearrange("(kt p) m -> p kt m", p=128))
            nc.sync.dma_start(bk_t[:], bk_d[:])

            # ---- persistent activation tiles ----
            qT = [actp.tile([128, LQ], F16, tag=f"qT{p}", name=f"qT{p}")
                  for p in range(2)]
            kT = [actp.tile([128, LQ], F16, tag=f"kT{p}", name=f"kT{p}")
                  for p in range(2)]
            v_t = actp.tile([128, NLK * VAW], F16, tag="v")
            aT = [actp.tile([128, LQ], F16, tag=f"aT{p}", name=f"aT{p}")
                  for p in range(2)]

            # constant ones (softmax denominator) / zero-pad rows of v_t
            v4 = v_t[:].rearrange("p (j w) -> p j w", w=VAW)
            for h in range(GH):
                nc.vector.memset(v4[:, :, VW * h + HD:VW * h + HD + ONES], 1.0)
                nc.vector.memset(v4[:, :, VW * h + HD + ONES:VW * (h + 1)], 0.0)

            # ---- K projection + interleaved weight DMAs, per 512-slice ----
            ct_tiles = {}
            for s in range(NS):
                for kt in range(KT_C):
                    t = ctp.tile([128, 512], F16, tag="ct")
                    nc.sync.dma_start(
                        t[:], cT_d[128 * kt:128 * (kt + 1), 512 * s:512 * (s + 1)])
                    ct_tiles[(kt, s)] = t
                for p in range(2):
                    ps = ps_w.tile([128, 512], F32, tag="mm")
                    for kt in range(KT_C):
                        nc.tensor.matmul(
                            ps[:], wk_t[:, 256 * kt + 128 * p:256 * kt + 128 * (p + 1)],
                            ct_tiles[(kt, s)][:],
                            start=(kt == 0), stop=(kt == KT_C - 1))
                    nc.scalar.activation(
                        kT[p][:, 512 * s:512 * (s + 1)], ps[:], IDENT,
                        bias=bk_t[:, p:p + 1])
                # stagger the remaining weight loads behind the ct slices
                if s == 0:
                    nc.sync.dma_start(
                        wv_t[:].rearrange("p (kt m) -> p kt m", m=GD),
                        wv_d.rearrange("(kt p) m -> p kt m", p=128))
                    nc.sync.dma_start(bvb_t[:], bvb_d[:])
                elif s == 1:
                    nc.sync.dma_start(
                        wq_t[:].rearrange("p (kt m) -> p kt m", m=256),
                        wq_d.rearrange("(kt p) m -> p kt m", p=128))
                    nc.sync.dma_start(bq_t[:], bq_d[:])
                elif s == 2:
                    nc.sync.dma_start(
                        wo_t[:].rearrange("p (p2 m) -> p p2 m", m=1024),
                        wo_d.rearrange("(p2 p) m -> p p2 m", p=128))

            # ---- V projection (ctx resident; x DMA still in flight) ----
            for j in range(NLK):
                ps = ps_w.tile([128, 512], F32, tag="mm")
                s, jj = j // 4, j % 4
                for kt in range(KT_C):
                    nc.tensor.matmul(
                        ps[:, 0:GD],
                        ct_tiles[(kt, s)][:, 128 * jj:128 * (jj + 1)],
                        wv_t[:, GD * kt:GD * (kt + 1)],
                        start=(kt == 0), stop=(kt == KT_C - 1))
                for h in range(GH):
                    nc.vector.tensor_add(
                        v_t[:, VAW * j + VW * h:VAW * j + VW * h + HD],
                        ps[:, HD * h:HD * (h + 1)],
                        bvb_t[:, HD * h:HD * (h + 1)])

            # ---- Q projection (both pairs), per 512-slice ----
            for s in range(NS):
                xt_tiles = []
                for kt in range(KT_Q):
                    t = xtp.tile([128, 512], F16, tag="xt")
                    nc.sync.dma_start(
                        t[:], xT_d[128 * kt:128 * (kt + 1), 512 * s:512 * (s + 1)])
                    xt_tiles.append(t)
                for p in range(2):
                    ps = ps_w.tile([128, 512], F32, tag="mm")
                    for kt in range(KT_Q):
                        nc.tensor.matmul(
                            ps[:], wq_t[:, 256 * kt + 128 * p:256 * kt + 128 * (p + 1)],
                            xt_tiles[kt][:],
                            start=(kt == 0), stop=(kt == KT_Q - 1))
                    nc.scalar.activation(
                        qT[p][:, 512 * s:512 * (s + 1)], ps[:], IDENT,
                        bias=bq_t[:, p:p + 1])

            def out_proj_group(mo, s):
                ops = ps_s.tile([128, 512], F32, tag="s", name=f"ops{mo}_{s}")
                for p in range(2):
                    nc.tensor.matmul(
                        ops[:], wo_t[:, D * p + 128 * mo:D * p + 128 * (mo + 1)],
                        aT[p][:, 512 * s:512 * (s + 1)],
                        start=(p == 0), stop=(p == 1))
                ot = outp.tile([128, 512], F16, tag="out")
                nc.vector.tensor_copy(ot[:], ops[:])
                nc.sync.dma_start(
                    out_d[128 * mo:128 * (mo + 1), 512 * s:512 * (s + 1)], ot[:])

            # ---- attention: q-slice outer; slice s-1's output projection
            # interleaved into slice s's j-loop to keep the exp stream dense
            for si in range(NS):
                cols = slice(512 * si, 512 * (si + 1))
                for p in range(2):
                    hA, hB = 2 * p, 2 * p + 1
                    pa = {hh: ps_w.tile([128, 512], F32, tag="mm",
                                        name=f"pa{si}_{p}_{hh}")
                          for hh in (0, 1)}
                    for j in range(NLK):
                        ks = slice(128 * j, 128 * (j + 1))
                        st = ps_s.tile([128, HALF], F32, tag="s")
                        # concurrent PE row-group pair: head A rows 0:64,
                        # head B rows 64:128, disjoint PSUM banks
                        nc.tensor.matmul(
                            st[:, 0:512], kT[p][0:64, ks], qT[p][0:64, cols],
                            start=True, stop=True)
                        nc.tensor.matmul(
                            st[:, 512:1024], kT[p][64:128, ks],
                            qT[p][64:128, cols], start=True, stop=True)
                        ex = expp.tile([128, HALF], F16, tag="expS")
                        nc.scalar.activation(ex[:], st[:], EXP, scale=SCALE)
                        nc.tensor.matmul(
                            pa[0][:],
                            v_t[:, VAW * j + VW * hA:VAW * j + VW * (hA + 1)],
                            ex[:, 0:512],
                            start=(j == 0), stop=(j == NLK - 1))
                        nc.tensor.matmul(
                            pa[1][:],
                            v_t[:, VAW * j + VW * hB:VAW * j + VW * (hB + 1)],
                            ex[:, 512:1024],
                            start=(j == 0), stop=(j == NLK - 1))
                        if si > 0 and j in (2, 6, 10, 14):
                            out_proj_group(4 * p + (j - 2) // 4, si - 1)
                    # normalize: attnT = attnU * (1/d); d-block replicated to
                    # 64 partitions via PSUM->SBUF shifts (SBUF->SBUF illegal)
                    for hh in (0, 1):
                        r0 = 64 * hh
                        pan = pa[hh]
                        dsb = rdp.tile([64, 512], F32, tag="dsb")
                        nc.vector.tensor_copy(dsb[0:32, :], pan[64:96, :])
                        nc.vector.tensor_copy(dsb[32:64, :], pan[64:96, :])
                        scr = scrp.tile([64, 512], F32, tag="scr")
                        nc.vector.tensor_copy(scr[:], pan[0:64, :])
                        rd = rdp.tile([64, 512], F32, tag="rd")
                        rds = rdp.tile([64, 512], F32, tag="rds")
                        nc.vector.reciprocal_approx_accurate(
                            rd[:], dsb[:], rds[:])
                        nc.vector.tensor_mul(
                            aT[p][r0:r0 + 64, cols], scr[:], rd[:])
            for mo in range(D // 128):
                out_proj_group(mo, NS - 1)

    nc.compile()
    return nc


_NC_CACHE = []


def _get_nc():
    if not _NC_CACHE:
        _NC_CACHE.append(_build())
    return _NC_CACHE[0]


def kernel_run(inputs, trace=False, **kw):
    """Run on HW; returns (full_output, BassKernelResults)."""
    x = np.asarray(inputs["x"], np.float32)
    context = np.asarray(inputs["context"], np.float32)
    w_q = np.asarray(inputs["w_q"], np.float32)
    b_q = np.asarray(inputs["b_q"], np.float32)
    w_k = np.asarray(inputs["w_k"], np.float32)
    b_k = np.asarray(inputs["b_k"], np.float32)
    w_v = np.asarray(inputs["w_v"], np.float32)
    b_v = np.asarray(inputs["b_v"], np.float32)
    w_o = np.asarray(inputs["w_o"], np.float32)
    b_o = np.asarray(inputs["b_o"], np.float32)

    f16 = np.float16
    xT_h = [np.ascontiguousarray(x[b].T).astype(f16) for b in range(B)]
    cT_h = [np.ascontiguousarray(context[b].T).astype(f16) for b in range(B)]

    maps = []
    for c in range(8):
        b, g = c // 4, c % 4
        hs = slice(256 * g, 256 * (g + 1))
        maps.append({
            "xT": xT_h[b],
            "ctxT": cT_h[b],
            "wq": np.ascontiguousarray(w_q[:, hs]).astype(f16),
            "wk": np.ascontiguousarray(w_k[:, hs]).astype(f16),
            "wv": np.ascontiguousarray(w_v[:, hs]).astype(f16),
            "wo": np.ascontiguousarray(w_o[hs, :]).astype(f16),
            "bq": np.ascontiguousarray(b_q[hs].reshape(2, 128).T),
            "bk": np.ascontiguousarray(b_k[hs].reshape(2, 128).T),
            "bvb": np.ascontiguousarray(
                np.broadcast_to(b_v[None, hs], (128, GD)).astype(np.float32)),
        })

    nc = _get_nc()
    res = bass_utils.run_bass_kernel_spmd(nc, maps, core_ids=list(range(8)),
                                          trace=trace, **kw)
    out = np.empty((B, LQ, D), np.float32)
    for b in range(B):
        acc = res.results[4 * b]["outT"].astype(np.float32)
        for g in range(1, 4):
            acc = acc + res.results[4 * b + g]["outT"].astype(np.float32)
        out[b] = acc.T + b_o[None, :]
    return out, res


def kernel(**inputs) -> np.ndarray:
    out, _ = kernel_run(inputs)
    return out


# revision 15
# speedup vs baseline: 1.3719x; 1.3717x over previous
"""Cross-attention Trainium2 kernel (nn_CrossAttention, B=2, L=2048, D=1024,
Dctx=768, 16 heads x 64).

Sharding: 8 cores = 2 (batch) x 4 (head-groups of 4 heads). Each core computes
its batch's Q/K/V projections for its 4 heads, flash-style attention in the
transposed (S^T) domain, and a partial output projection; the host sums the
head-group partials and adds b_o.

All activations live transposed on-chip (xT, ctxT, qT, kT, attnT) so every
matmul contracts over the partition dim with no on-chip transposes; operands
are fp16 (full PE streaming rate) with fp32 PSUM accumulation. Heads are
processed in pairs: the pair's scores matmuls contract K=64 on PE row-groups
(0,0) and (64,0) and stream CONCURRENTLY into the two banks of one [128,1024]
PSUM tile, so a head-pair's scores cost one stream instead of two. One
1024-wide exp covers both heads. V tiles are padded to 128 columns (64 v + 32
ones for the softmax denominator + 32 zeros) so every stationary load takes
the fast-weight-load path. Output partials are fp16, summed on the host.
"""
import numpy as np

import concourse.bass as bass
import concourse.tile as tile
from concourse import bacc, mybir, bass_utils

F16 = mybir.dt.float16
F32 = mybir.dt.float32
EXP = mybir.ActivationFunctionType.Exp
IDENT = mybir.ActivationFunctionType.Identity

# Problem shape (hardcoded per harness contract)
B, LQ, D = 2, 2048, 1024
DCTX = 768
NH, HD = 16, 64
SCALE = 1.0 / 8.0  # 1/sqrt(64)

# Per-core shard: 4 heads (one group), one batch
GH = 4                # heads per core
ONES = 32             # d-replication rows per head
VW = 128              # per-head v_t width: 64 v + 32 ones + 32 zero pad (FWL)
VAW = GH * VW         # 512
GD = GH * HD          # 256: real v columns per chunk
KT_Q = D // 128       # 8
KT_C = DCTX // 128    # 6
NLK = LQ // 128       # 16 key tiles
NS = LQ // 512        # 4 query 512-slices
HALF = 1024


def _build():
    nc = bacc.Bacc("TRN2", target_bir_lowering=False, debug=False,
                   enable_asserts=False, num_devices=8)

    xT_d = nc.dram_tensor("xT", (D, LQ), F16, kind="ExternalInput").ap()
    cT_d = nc.dram_tensor("ctxT", (DCTX, LQ), F16, kind="ExternalInput").ap()
    wq_d = nc.dram_tensor("wq", (D, 256), F16, kind="ExternalInput").ap()
    wk_d = nc.dram_tensor("wk", (DCTX, 256), F16, kind="ExternalInput").ap()
    wv_d = nc.dram_tensor("wv", (DCTX, GD), F16, kind="ExternalInput").ap()
    wo_d = nc.dram_tensor("wo", (256, D), F16, kind="ExternalInput").ap()
    bq_d = nc.dram_tensor("bq", (128, 2), F32, kind="ExternalInput").ap()
    bk_d = nc.dram_tensor("bk", (128, 2), F32, kind="ExternalInput").ap()
    bvb_d = nc.dram_tensor("bvb", (128, GD), F32, kind="ExternalInput").ap()
    out_d = nc.dram_tensor("outT", (D, LQ), F16, kind="ExternalOutput").ap()

    with tile.TileContext(nc) as tc:
        with tc.tile_pool(name="w", bufs=1) as wp, \
             tc.tile_pool(name="xt", bufs=10) as xtp, \
             tc.tile_pool(name="ct", bufs=24) as ctp, \
             tc.tile_pool(name="act", bufs=1) as actp, \
             tc.tile_pool(name="expp", bufs=3) as expp, \
             tc.tile_pool(name="scrp", bufs=3) as scrp, \
             tc.tile_pool(name="rdp", bufs=3) as rdp, \
             tc.tile_pool(name="outp", bufs=3) as outp, \
             tc.tile_pool(name="ps_w", bufs=2, space="PSUM") as ps_w, \
             tc.tile_pool(name="ps_s", bufs=2, space="PSUM") as ps_s, \
             tc.tile_pool(name="ps_w2", bufs=1, space="PSUM") as ps_w2, \
             tc.tile_pool(name="ps_o", bufs=1, space="PSUM") as ps_o:

            # ---- weight/bias tiles (DMAs issued interleaved below) ----
            wq_t = wp.tile([128, KT_Q * 256], F16, tag="wq")
            wk_t = wp.tile([128, KT_C * 256], F16, tag="wk")
            wv_t = wp.tile([128, KT_C * GD], F16, tag="wv")
            wo_t = wp.tile([128, 2 * D], F16, tag="wo")
            bq_t = wp.tile([128, 2], F32, tag="bq")
            bk_t = wp.tile([128, 2], F32, tag="bk")
            bvb_t = wp.tile([128, GD], F32, tag="bvb")

            # K proj needs these first
            nc.sync.dma_start(wk_t[:].rearrange("p (kt m) -> p kt m", m=256),
                              wk_d.rearrange("(kt p) m -> p kt m", p=128))
            nc.sync.dma_start(bk_t[:], bk_d[:])

            # ---- persistent activation tiles ----
            qT = [actp.tile([128, LQ], F16, tag=f"qT{p}", name=f"qT{p}")
                  for p in range(2)]
            kT = [actp.tile([128, LQ], F16, tag=f"kT{p}", name=f"kT{p}")
                  for p in range(2)]
            v_t = actp.tile([128, NLK * VAW], F16, tag="v")
            aT = [actp.tile([128, LQ], F16, tag=f"aT{p}", name=f"aT{p}")
                  for p in range(2)]

            # constant ones (softmax denominator) / zero-pad rows of v_t
            v4 = v_t[:].rearrange("p (j w) -> p j w", w=VAW)
            for h in range(GH):
                nc.vector.memset(v4[:, :, VW * h + HD:VW * h + HD + ONES], 1.0)
                nc.vector.memset(v4[:, :, VW * h + HD + ONES:VW * (h + 1)], 0.0)

            # ---- K projection + interleaved weight DMAs, per 512-slice ----
            ct_tiles = {}
            for s in range(NS):
                for kt in range(KT_C):
                    t = ctp.tile([128, 512], F16, tag="ct")
                    nc.sync.dma_start(
                        t[:], cT_d[128 * kt:128 * (kt + 1), 512 * s:512 * (s + 1)])
                    ct_tiles[(kt, s)] = t
                for p in range(2):
                    ps = ps_w.tile([128, 512], F32, tag="mm")
                    for kt in range(KT_C):
                        nc.tensor.matmul(
                            ps[:], wk_t[:, 256 * kt + 128 * p:256 * kt + 128 * (p + 1)],
                            ct_tiles[(kt, s)][:],
                            start=(kt == 0), stop=(kt == KT_C - 1))
                    nc.scalar.activation(
                        kT[p][:, 512 * s:512 * (s + 1)], ps[:], IDENT,
                        bias=bk_t[:, p:p + 1])
                # stagger the remaining weight loads behind the ct slices
                if s == 0:
                    nc.sync.dma_start(
                        wv_t[:].rearrange("p (kt m) -> p kt m", m=GD),
                        wv_d.rearrange("(kt p) m -> p kt m", p=128))
                    nc.sync.dma_start(bvb_t[:], bvb_d[:])
                elif s == 1:
                    nc.sync.dma_start(
                        wq_t[:].rearrange("p (kt m) -> p kt m", m=256),
                        wq_d.rearrange("(kt p) m -> p kt m", p=128))
                    nc.sync.dma_start(bq_t[:], bq_d[:])
                elif s == 2:
                    nc.sync.dma_start(
                        wo_t[:].rearrange("p (p2 m) -> p p2 m", m=1024),
                        wo_d.rearrange("(p2 p) m -> p p2 m", p=128))

            # ---- V projection (ctx resident; x DMA still in flight) ----
            for j in range(NLK):
                ps = ps_w.tile([128, 512], F32, tag="mm")
                s, jj = j // 4, j % 4
                for kt in range(KT_C):
                    nc.tensor.matmul(
                        ps[:, 0:GD],
                        ct_tiles[(kt, s)][:, 128 * jj:128 * (jj + 1)],
                        wv_t[:, GD * kt:GD * (kt + 1)],
                        start=(kt == 0), stop=(kt == KT_C - 1))
                for h in range(GH):
                    nc.vector.tensor_add(
                        v_t[:, VAW * j + VW * h:VAW * j + VW * h + HD],
                        ps[:, HD * h:HD * (h + 1)],
                        bvb_t[:, HD * h:HD * (h + 1)])

            # ---- Q projection; slice 0 up front, rest interleaved into
            # attention (q-slice si+1 computed during attention on si)
            def q_proj(s, pp):
                xt_tiles = []
                for kt in range(KT_Q):
                    t = xtp.tile([128, 512], F16, tag="xt",
                                 name=f"xt{s}_{pp}_{kt}")
                    nc.sync.dma_start(
                        t[:], xT_d[128 * kt:128 * (kt + 1), 512 * s:512 * (s + 1)])
                    xt_tiles.append(t)
                ps = ps_o.tile([128, 512], F32, tag="o", name=f"qps{s}_{pp}")
                for kt in range(KT_Q):
                    nc.tensor.matmul(
                        ps[:], wq_t[:, 256 * kt + 128 * pp:256 * kt + 128 * (pp + 1)],
                        xt_tiles[kt][:],
                        start=(kt == 0), stop=(kt == KT_Q - 1))
                nc.scalar.activation(
                    qT[pp][:, 512 * s:512 * (s + 1)], ps[:], IDENT,
                    bias=bq_t[:, pp:pp + 1])

            for pp in range(2):
                q_proj(0, pp)

            def out_proj_group(mo, s):
                ops = ps_o.tile([128, 512], F32, tag="o", name=f"ops{mo}_{s}")
                for p in range(2):
                    nc.tensor.matmul(
                        ops[:], wo_t[:, D * p + 128 * mo:D * p + 128 * (mo + 1)],
                        aT[p][:, 512 * s:512 * (s + 1)],
                        start=(p == 0), stop=(p == 1))
                ot = outp.tile([128, 512], F16, tag="out")
                nc.vector.tensor_copy(ot[:], ops[:])
                nc.sync.dma_start(
                    out_d[128 * mo:128 * (mo + 1), 512 * s:512 * (s + 1)], ot[:])

            # ---- attention: q-slice outer; slice s-1's output projection
            # interleaved into slice s's j-loop to keep the exp stream dense
            for si in range(NS):
                cols = slice(512 * si, 512 * (si + 1))
                for p in range(2):
                    hA, hB = 2 * p, 2 * p + 1
                    pa = {0: ps_w.tile([128, 512], F32, tag="mm",
                                       name=f"pa{si}_{p}_0"),
                          1: ps_w2.tile([128, 512], F32, tag="mm2",
                                        name=f"pa{si}_{p}_1")}
                    for j in range(NLK):
                        ks = slice(128 * j, 128 * (j + 1))
                        st = ps_s.tile([128, HALF], F32, tag="s")
                        # concurrent PE row-group pair: head A rows 0:64,
                        # head B rows 64:128, disjoint PSUM banks
                        nc.tensor.matmul(
                            st[:, 0:512], kT[p][0:64, ks], qT[p][0:64, cols],
                            start=True, stop=True)
                        nc.tensor.matmul(
                            st[:, 512:1024], kT[p][64:128, ks],
                            qT[p][64:128, cols], start=True, stop=True)
                        ex = expp.tile([128, HALF], F16, tag="expS")
                        nc.scalar.activation(ex[:], st[:], EXP, scale=SCALE)
                        nc.tensor.matmul(
                            pa[0][:],
                            v_t[:, VAW * j + VW * hA:VAW * j + VW * (hA + 1)],
                            ex[:, 0:512],
                            start=(j == 0), stop=(j == NLK - 1))
                        nc.tensor.matmul(
                            pa[1][:],
                            v_t[:, VAW * j + VW * hB:VAW * j + VW * (hB + 1)],
                            ex[:, 512:1024],
                            start=(j == 0), stop=(j == NLK - 1))
                        if si > 0 and j in (2, 6, 10, 14):
                            out_proj_group(4 * p + (j - 2) // 4, si - 1)
                        if si < NS - 1 and j == NLK - 1:
                            q_proj(si + 1, p)
                    # normalize: attnT = attnU * (1/d); d-block replicated to
                    # 64 partitions via PSUM->SBUF shifts (SBUF->SBUF illegal)
                    for hh in (0, 1):
                        r0 = 64 * hh
                        pan = pa[hh]
                        dsb = rdp.tile([64, 512], F32, tag="dsb")
                        nc.vector.tensor_copy(dsb[0:32, :], pan[64:96, :])
                        nc.vector.tensor_copy(dsb[32:64, :], pan[64:96, :])
                        scr = scrp.tile([64, 512], F32, tag="scr")
                        nc.vector.tensor_copy(scr[:], pan[0:64, :])
                        rd = rdp.tile([64, 512], F32, tag="rd")
                        rds = rdp.tile([64, 512], F32, tag="rds")
                        nc.vector.reciprocal_approx_accurate(
                            rd[:], dsb[:], rds[:])
                        nc.vector.tensor_mul(
                            aT[p][r0:r0 + 64, cols], scr[:], rd[:])
            for mo in range(D // 128):
                out_proj_group(mo, NS - 1)

    nc.compile()
    return nc


_NC_CACHE = []


def _get_nc():
    if not _NC_CACHE:
        _NC_CACHE.append(_build())
    return _NC_CACHE[0]


def kernel_run(inputs, trace=False, **kw):
    """Run on HW; returns (full_output, BassKernelResults)."""
    x = np.asarray(inputs["x"], np.float32)
    context = np.asarray(inputs["context"], np.float32)
    w_q = np.asarray(inputs["w_q"], np.float32)
    b_q = np.asarray(inputs["b_q"], np.float32)
    w_k = np.asarray(inputs["w_k"], np.float32)
    b_k = np.asarray(inputs["b_k"], np.float32)
    w_v = np.asarray(inputs["w_v"], np.float32)
    b_v = np.asarray(inputs["b_v"], np.float32)
    w_o = np.asarray(inputs["w_o"], np.float32)
    b_o = np.asarray(inputs["b_o"], np.float32)

    f16 = np.float16
    xT_h = [np.ascontiguousarray(x[b].T).astype(f16) for b in range(B)]
    cT_h = [np.ascontiguousarray(context[b].T).astype(f16) for b in range(B)]

    maps = []
    for c in range(8):
        b, g = c // 4, c % 4
        hs = slice(256 * g, 256 * (g + 1))
        maps.append({
            "xT": xT_h[b],
            "ctxT": cT_h[b],
            "wq": np.ascontiguousarray(w_q[:, hs]).astype(f16),
            "wk": np.ascontiguousarray(w_k[:, hs]).astype(f16),
            "wv": np.ascontiguousarray(w_v[:, hs]).astype(f16),
            "wo": np.ascontiguousarray(w_o[hs, :]).astype(f16),
            "bq": np.ascontiguousarray(b_q[hs].reshape(2, 128).T),
            "bk": np.ascontiguousarray(b_k[hs].reshape(2, 128).T),
            "bvb": np.ascontiguousarray(
                np.broadcast_to(b_v[None, hs], (128, GD)).astype(np.float32)),
        })

    nc = _get_nc()
    res = bass_utils.run_bass_kernel_spmd(nc, maps, core_ids=list(range(8)),
                                          trace=trace, **kw)
    out = np.empty((B, LQ, D), np.float32)
    for b in range(B):
        acc = res.results[4 * b]["outT"].astype(np.float32)
        for g in range(1, 4):
            acc = acc + res.results[4 * b + g]["outT"].astype(np.float32)
        out[b] = acc.T + b_o[None, :]
    return out, res


def kernel(**inputs) -> np.ndarray:
    out, _ = kernel_run(inputs)
    return out
